# revision 9
# baseline (speedup 1.0000x reference)
"""Trainium2 Bass kernel for nn_DRQ_2448131359005 (residual VQ, M=4 stages).

Strategy (per NeuronCore, data-parallel over N):
  - residual r kept in n-layout tiles [128, 32]
  - per stage, per 512-row group:
      raug [34, 512] = [2s*r^T ; -s^2 ; -rowmax]  (PE transpose + ACT scale)
      l    [128n, 2048k] = raug[0:33].T @ cbT_aug[0:33]  (PE, fp32)  -> argmax path
      DVE: tensor_tensor_reduce evac (l -> SBUF, rowmax), max_index -> codes
      hard = cb_s[code] via GPSIMD indirect DMA gather from DRAM
      l^T - m [128k, 512n] = cbT_aug[:, chunk].T @ raug  (PE)  -> exp (ACT, bf16)
      num/den = [cb|1].T @ E^T chunks (PE accumulate)
      soft = s * num / den applied in n-layout (DVE per-partition scalars)
  - SSE partials per partition -> host sums across partitions/cores for the loss.

kernel(x, codebook, scale) -> (hard_codes [N,4] int32, loss f32 scalar)
"""

import os
import numpy as np
from contextlib import ExitStack

import concourse.bass as bass
import concourse.bacc as bacc_mod
import concourse.mybir as mybir
from concourse.tile import TileContext
from concourse.masks import make_identity
from concourse.bass_utils import run_bass_kernel_spmd

F32 = mybir.dt.float32
BF16 = mybir.dt.bfloat16
U32 = mybir.dt.uint32
I32 = mybir.dt.int32
Alu = mybir.AluOpType
ActF = mybir.ActivationFunctionType

N, D, K, M = 131072, 32, 2048, 4
N_CORES = 8
P = 128
KC = K // P          # 16 k-chunks
GT = 4               # n-tiles per group
FB = P * GT          # 512 free block
FLT_MIN = -3.4e38


def _build(nl):
    """Build the Bass program for one core processing nl rows."""
    nt = nl // P
    ng = nt // GT
    nc = bacc_mod.Bacc("TRN2", target_bir_lowering=False)

    x_d = nc.declare_dram_parameter("x", [nl, D], F32, isOutput=False)
    cb_d = nc.declare_dram_parameter("codebook", [K, D], F32, isOutput=False)
    sc_d = nc.declare_dram_parameter("scale", [M], F32, isOutput=False)
    codes_d = nc.declare_dram_parameter("codes", [nl, M], I32, isOutput=True)
    sse_d = nc.declare_dram_parameter("sse", [P, 12], F32, isOutput=True)
    cbs_ds = [nc.dram_tensor(f"cbs{i}", [K, D], F32) for i in range(M)]

    dbg_stages = int(os.environ.get("KDBG_STAGES", "4"))
    dbg_groups = int(os.environ.get("KDBG_GROUPS", "9999"))
    dbg_no_gather = os.environ.get("KDBG_NO_GATHER", "0") == "1"
    dbg_no_maxidx = os.environ.get("KDBG_NO_MAXIDX", "0") == "1"
    dbg_no_phase_e = os.environ.get("KDBG_NO_E", "0") == "1"
    with TileContext(nc) as tc, ExitStack() as ctx:
        cp = ctx.enter_context(tc.tile_pool(name="const", bufs=1))
        st = ctx.enter_context(tc.tile_pool(name="state", bufs=1))
        rg = ctx.enter_context(tc.tile_pool(name="raug", bufs=2))
        etp = ctx.enter_context(tc.tile_pool(name="et", bufs=2))
        nsb = ctx.enter_context(tc.tile_pool(name="numsb", bufs=2))
        sm = ctx.enter_context(tc.tile_pool(name="small", bufs=10))
        ps_l = ctx.enter_context(tc.tile_pool(name="ps_l", bufs=1, space="PSUM"))
        ps_lt = ctx.enter_context(tc.tile_pool(name="ps_lt", bufs=2, space="PSUM"))
        ps_num = ctx.enter_context(tc.tile_pool(name="ps_num", bufs=1, space="PSUM"))
        ps_ms = ctx.enter_context(tc.tile_pool(name="ps_ms", bufs=1, space="PSUM"))

        # ---------------- constants / preamble ----------------
        ident = cp.tile([P, P], F32)
        make_identity(nc, ident[:])

        cb_sb = cp.tile([P, KC, D], F32)          # row k = c*128+p at [p, c, :]
        nc.sync.dma_start(out=cb_sb[:], in_=cb_d[:].rearrange("(c p) d -> p c d", p=P))

        s_raw = cp.tile([1, M], F32)
        nc.sync.dma_start(out=s_raw[:], in_=sc_d[:].rearrange("(a m) -> a m", a=1))
        s_all = cp.tile([P, M], F32)
        nc.gpsimd.partition_broadcast(s_all[:], s_raw[0:1, :])
        s2_all = cp.tile([P, M], F32)
        nc.vector.tensor_scalar(s2_all[:], s_all[:], 2.0, None, op0=Alu.mult)
        sneg_all = cp.tile([P, M], F32)   # -s^2
        nc.vector.scalar_tensor_tensor(
            sneg_all[:], s_all[:], -1.0, s_all[:], op0=Alu.mult, op1=Alu.mult
        )

        cb_bf = cp.tile([P, KC, D + 1], BF16)     # [cb | 1] lhsT chunks
        nc.vector.tensor_copy(cb_bf[:, :, 0:D], cb_sb[:])
        nc.vector.memset(cb_bf[:, :, D : D + 1], 1.0)

        ones8 = cp.tile([P, 8], F32)
        nc.vector.memset(ones8[:], 1.0)
        ones_row = cp.tile([1, FB], F32)
        nc.vector.memset(ones_row[:], 1.0)
        ones_c32 = cp.tile([D, 1], F32)
        nc.vector.memset(ones_c32[:], 1.0)

        # cbT_aug [rows 0-31: cb^T, row 32: ||c||^2, row 33: 1] + replica at 64..97
        # (engine APs need 32-aligned partition bases: preset rows 32-63 to 1.0,
        #  then overwrite row 32 with ||c||^2.)
        cbt = cp.tile([P, K], F32)
        nc.vector.memset(cbt[D : 64, :], 1.0)
        for c in range(KC):
            tp = ps_ms.tile([D, P], F32, tag="ms")
            nc.tensor.transpose(tp[:], cb_sb[:, c, :], ident[:])
            nc.scalar.copy(cbt[0:D, c * P : (c + 1) * P], tp[:])
        cbt2 = cp.tile([D, K], F32)
        nc.vector.tensor_tensor(
            out=cbt2[:], in0=cbt[0:D, :], in1=cbt[0:D, :], op=Alu.mult
        )
        for q in range(K // FB):
            c2p = ps_ms.tile([1, FB], F32, tag="ms")
            nc.tensor.matmul(
                c2p[:], ones_c32[:], cbt2[:, q * FB : (q + 1) * FB],
                start=True, stop=True,
            )
            nc.scalar.copy(cbt[D : D + 1, q * FB : (q + 1) * FB], c2p[:])
        nc.sync.dma_start(out=cbt[64 : 64 + D + 2, :], in_=cbt[0 : D + 2, :])

        # scaled codebooks to DRAM for the hard-path gather
        for i in range(M):
            cbs_sb = sm.tile([P, KC, D], F32, tag="cbs_tmp")
            nc.vector.tensor_scalar(
                cbs_sb[:], cb_sb[:], s_all[:, i : i + 1], None, op0=Alu.mult
            )
            nc.sync.dma_start(
                out=cbs_ds[i][:].rearrange("(c p) d -> p c d", p=P), in_=cbs_sb[:]
            )

        # ---------------- state ----------------
        x_all = st.tile([P, nt, D], F32)
        nc.sync.dma_start(out=x_all[:], in_=x_d[:].rearrange("(t p) d -> p t d", p=P))
        r_all = st.tile([P, nt, D], F32)
        nc.vector.tensor_copy(r_all[:], x_all[:])
        qs_all = st.tile([P, nt, D], F32)
        nc.gpsimd.memset(qs_all[:], 0.0)
        qh_all = st.tile([P, nt, D], F32)
        nc.gpsimd.memset(qh_all[:], 0.0)
        codes_all = st.tile([P, nt, M], U32)
        sse_all = st.tile([P, 9, nt], F32)

        # ---------------- stages ----------------
        for i in range(min(M, dbg_stages)):
            s_i = s_all[:, i : i + 1]
            s2_i = s2_all[:, i : i + 1]
            sneg_i = sneg_all[:, i : i + 1]
            for g in range(min(ng, dbg_groups)):
                t0 = g * GT
                # ---- phase A: raug rows 0-32 ----
                raug = rg.tile([P, FB], F32, tag="raug")
                trp = ps_ms.tile([D, FB], F32, tag="ms")
                for j in range(GT):
                    nc.tensor.transpose(
                        trp[:, j * P : (j + 1) * P], r_all[:, t0 + j, :], ident[:]
                    )
                nc.scalar.mul(raug[0:D, :], trp[:], s2_i[0:D, :])
                nc.vector.tensor_scalar(
                    raug[D : D + 1, :], ones_row[:], sneg_i[0:1, :], None, op0=Alu.mult
                )
                nc.sync.dma_start(
                    out=raug[64 : 64 + D + 1, :], in_=raug[0 : D + 1, :]
                )

                # ---- phase B: l (n-layout), argmax, gather ----
                m4 = sm.tile([P, GT], F32, tag="m4")
                hard4 = sm.tile([P, GT, D], F32, tag="hard4")
                for j in range(GT):
                    base = 64 if (j % 2) else 0
                    l_ps = ps_l.tile([P, K], F32, tag="l")
                    lhsT = raug[base : base + D + 1, j * P : (j + 1) * P]
                    for q in range(K // FB):
                        nc.tensor.matmul(
                            l_ps[:, q * FB : (q + 1) * FB],
                            lhsT,
                            cbt[base : base + D + 1, q * FB : (q + 1) * FB],
                            start=True, stop=True,
                        )
                    nc.vector.tensor_reduce(
                        out=m4[:, j : j + 1], in_=l_ps[:],
                        axis=mybir.AxisListType.X, op=Alu.max,
                    )
                    mb8 = sm.tile([P, 8], F32, tag="mb8")
                    nc.vector.tensor_scalar(
                        mb8[:], ones8[:], m4[:, j : j + 1], None, op0=Alu.mult
                    )
                    idx8 = sm.tile([P, 8], U32, tag="idx8")
                    if dbg_no_maxidx:
                        nc.vector.memset(idx8[:], 0)
                    else:
                        nc.vector.max_index(idx8[:], mb8[:], l_ps[:])
                    nc.vector.tensor_copy(
                        codes_all[:, t0 + j, i : i + 1], idx8[:, 0:1]
                    )
                    if dbg_no_gather:
                        nc.vector.memset(hard4[:, j, :], 0.0)
                    else:
                        nc.gpsimd.indirect_dma_start(
                            out=hard4[:, j, :],
                            out_offset=None,
                            in_=cbs_ds[i][:],
                            in_offset=bass.IndirectOffsetOnAxis(ap=idx8[:, 0:1], axis=0),
                        )

                # ---- phase C: n-layout hard updates ----
                for j in range(GT):
                    t = t0 + j
                    nc.vector.tensor_tensor(
                        out=r_all[:, t, :], in0=r_all[:, t, :], in1=hard4[:, j, :],
                        op=Alu.subtract,
                    )
                    nc.vector.tensor_tensor(
                        out=qh_all[:, t, :], in0=qh_all[:, t, :], in1=hard4[:, j, :],
                        op=Alu.add,
                    )
                    dif = sm.tile([P, D], F32, tag="dif")
                    nc.vector.tensor_tensor(
                        out=dif[:], in0=x_all[:, t, :], in1=qh_all[:, t, :],
                        op=Alu.subtract,
                    )
                    nc.vector.tensor_tensor(
                        out=dif[:], in0=dif[:], in1=dif[:], op=Alu.mult,
                    )
                    nc.vector.tensor_reduce(
                        out=sse_all[:, 4 + i, t : t + 1], in_=dif[:],
                        axis=mybir.AxisListType.X, op=Alu.add,
                    )

                # ---- phase D: raug row 33 = -m (staged via base-0 scratch row) ----
                mrow = sm.tile([1, FB], F32, tag="mrow")
                for j in range(GT):
                    mtp = ps_ms.tile([1, P], F32, tag="ms")
                    nc.tensor.transpose(mtp[:], m4[:, j : j + 1], ident[:])
                    nc.scalar.mul(mrow[0:1, j * P : (j + 1) * P], mtp[:], -1.0)
                nc.sync.dma_start(out=raug[D + 1 : D + 2, :], in_=mrow[:])
                nc.sync.dma_start(
                    out=raug[64 + D + 1 : 64 + D + 2, :], in_=mrow[:]
                )

                if dbg_no_phase_e:
                    continue
                # ---- phase E: l^T - m, exp, num/den ----
                et = etp.tile([P, KC, FB], BF16, tag="et")
                num_ps = ps_num.tile([D + 1, FB], F32, tag="num")
                for c in range(KC):
                    base = 64 if (c % 2) else 0
                    lt_ps = ps_lt.tile([P, FB], F32, tag="lt")
                    nc.tensor.matmul(
                        lt_ps[:],
                        cbt[base : base + D + 2, c * P : (c + 1) * P],
                        raug[base : base + D + 2, :],
                        start=True, stop=True,
                    )
                    nc.scalar.activation(et[:, c, :], lt_ps[:], ActF.Exp)
                    nc.tensor.matmul(
                        num_ps[:],
                        cb_bf[:, c, :],
                        et[:, c, :],
                        start=(c == 0), stop=(c == KC - 1),
                    )
                num_sb = nsb.tile([D + 1, FB], F32, tag="numsb")
                nc.scalar.copy(num_sb[:], num_ps[:])

                for j in range(GT):
                    t = t0 + j
                    ntp = ps_ms.tile([P, D + 1], F32, tag="ms")
                    nc.tensor.transpose(
                        ntp[:], num_sb[:, j * P : (j + 1) * P],
                        ident[0 : D + 1, 0 : D + 1],
                    )
                    recip = sm.tile([P, 1], F32, tag="recip")
                    nc.vector.reciprocal(recip[:], ntp[:, D : D + 1])
                    sxr = sm.tile([P, 1], F32, tag="sxr")
                    nc.vector.tensor_scalar(
                        sxr[:], recip[:], s_i, None, op0=Alu.mult
                    )
                    nc.vector.scalar_tensor_tensor(
                        out=qs_all[:, t, :], in0=ntp[:, 0:D], scalar=sxr[:],
                        in1=qs_all[:, t, :], op0=Alu.mult, op1=Alu.add,
                    )
                    dif = sm.tile([P, D], F32, tag="dif")
                    nc.vector.tensor_tensor(
                        out=dif[:], in0=x_all[:, t, :], in1=qs_all[:, t, :],
                        op=Alu.subtract,
                    )
                    nc.vector.tensor_tensor(
                        out=dif[:], in0=dif[:], in1=dif[:], op=Alu.mult,
                    )
                    nc.vector.tensor_reduce(
                        out=sse_all[:, i, t : t + 1], in_=dif[:],
                        axis=mybir.AxisListType.X, op=Alu.add,
                    )
                    if i == M - 1:
                        difj = sm.tile([P, D], F32, tag="dif")
                        nc.vector.tensor_tensor(
                            out=difj[:], in0=qs_all[:, t, :], in1=qh_all[:, t, :],
                            op=Alu.subtract,
                        )
                        nc.vector.tensor_tensor(
                            out=difj[:], in0=difj[:], in1=difj[:], op=Alu.mult,
                        )
                        nc.vector.tensor_reduce(
                            out=sse_all[:, 8, t : t + 1], in_=difj[:],
                            axis=mybir.AxisListType.X, op=Alu.add,
                        )

        # ---------------- outputs ----------------
        sse_out = st.tile([P, 12], F32)
        nc.vector.memset(sse_out[:], 0.0)
        for q in range(9):
            nc.vector.tensor_reduce(
                out=sse_out[:, q : q + 1], in_=sse_all[:, q, :],
                axis=mybir.AxisListType.X, op=Alu.add,
            )
        nc.sync.dma_start(out=sse_d[:], in_=sse_out[:])
        nc.sync.dma_start(
            out=codes_d[:].rearrange("(t p) m -> p t m", p=P),
            in_=codes_all[:].bitcast(I32),
        )

    nc.finalize()
    return nc


_NC_CACHE = {}


def _get_nc(nl):
    if nl not in _NC_CACHE:
        _NC_CACHE[nl] = _build(nl)
    return _NC_CACHE[nl]


def kernel(x, codebook, scale):
    x = np.ascontiguousarray(np.asarray(x, dtype=np.float32))
    codebook = np.ascontiguousarray(np.asarray(codebook, dtype=np.float32))
    scale = np.ascontiguousarray(np.asarray(scale, dtype=np.float32))
    n = x.shape[0]
    nl = n // N_CORES
    nc = _get_nc(nl)
    in_maps = [
        {
            "x": x[c * nl : (c + 1) * nl],
            "codebook": codebook,
            "scale": scale,
        }
        for c in range(N_CORES)
    ]
    res = run_bass_kernel_spmd(nc, in_maps, list(range(N_CORES))).results
    codes = np.concatenate([res[c]["codes"] for c in range(N_CORES)], axis=0)
    sse = np.stack([res[c]["sse"] for c in range(N_CORES)])  # [8, 128, 12]
    tot = sse.sum(axis=(0, 1), dtype=np.float32)             # [12]
    nd = np.float32(n * D)
    soft_dist = np.float32(tot[0:4].sum() / nd)
    hard_dist = np.float32(tot[4:8].sum() / nd)
    joint = np.float32(tot[8] / nd)
    loss = np.float32(0.1) * soft_dist + hard_dist + np.float32(0.1) * joint
    return codes, np.float32(loss)
